# revision 1
# baseline (speedup 1.0000x reference)
"""Trainium2 Bass kernel for GNN message passing (nn_MessageModel).

Reference computation:
    inp = concat([x[col], edge_attr], 1)          # [E, 48]
    h = relu(inp @ W1 + b1)                       # [E, 64]
    messages = h @ W2 + b2                        # [E, 32]
    out = segment_sum(messages, row, N)           # [N, 32]

Strategy (8 NeuronCores, SPMD):
- Host: sort edges by destination row; shard by destination node range
  (12500 nodes/core) so per-core outputs are disjoint (no collective
  needed). Build feature-major inp^T tiles (x[col] rows + edge_attr,
  transposed) in sorted edge order, packed two 512-edge "lanes" per
  96-partition rhs tile.
- HW per 2048-edge supergroup (4 lane-packed 512-edge node-aligned
  groups): one DMA loads inp^T [96, 1024]; per pair of lanes one
  block-diagonal W1 matmul -> h_pre PSUM [128, 512]; DVE bias+relu;
  block-diagonal W2 matmul -> msg^T PSUM [128, 512] (4 lanes x 32
  features); DVE cumsum scan along edges; GPSIMD ap_gather extracts
  each node's last-edge cumsum column; DMA out.
- Host: per-node adjacent differences of the extracted cumsums, add
  deg * b2, assemble [N, 32].
"""
import sys

if "/opt/trn_rl_repo" not in sys.path:
    sys.path.insert(0, "/opt/trn_rl_repo")

import numpy as np
import ml_dtypes

BF16 = ml_dtypes.bfloat16

N_NODES = 100000
N_EDGES = 1600000
D_NODE = 32
D_EDGE = 16
D_IN = D_NODE + D_EDGE
D_HID = 64
D_OUT = 32

N_CORES = 8
NPC = N_NODES // N_CORES          # nodes per core
P = 128
GE = 512                          # edges per scan group (lane)
EXT_BATCH = 8                     # supergroups of ext output per out-DMA
LANES = 4                         # lanes per supergroup
SGE = GE * LANES                  # edges per supergroup
NODE_SLOTS = 64                   # max nodes per group
EDGE_CAP = 464                    # grouping edge budget per group
NODE_CAP = 56                     # grouping node budget per group

_compiled_cache = {}


# ----------------------------------------------------------------------------
# host-side preprocessing
# ----------------------------------------------------------------------------

def _preprocess(x, edge_index, edge_attr, W1, b1, W2, b2):
    x = np.asarray(x, dtype=np.float32)
    W1 = np.asarray(W1, dtype=np.float32)
    W2 = np.asarray(W2, dtype=np.float32)
    row = np.asarray(edge_index[0], dtype=np.int64)
    col = np.asarray(edge_index[1], dtype=np.int64)
    order = np.argsort(row, kind="stable")
    col_s = col[order].astype(np.int64)
    attr_s = np.asarray(edge_attr, dtype=np.float32)[order]

    deg = np.bincount(row, minlength=N_NODES).astype(np.int64)
    cum_deg = np.zeros(N_NODES + 1, dtype=np.int64)
    np.cumsum(deg, out=cum_deg[1:])

    cores = []
    max_groups = 0
    for k in range(N_CORES):
        n0, n1 = k * NPC, (k + 1) * NPC
        e0, e1 = cum_deg[n0], cum_deg[n1]
        d_k = deg[n0:n1]
        node_e0 = cum_deg[n0:n1] - e0
        g1 = node_e0 // EDGE_CAP
        g2 = np.arange(NPC) // NODE_CAP
        _, g = np.unique(np.maximum(g1, g2), return_inverse=True)
        n_groups = int(g[-1]) + 1
        grp_node_start = np.searchsorted(g, np.arange(n_groups), side="left")
        grp_node_end = np.searchsorted(g, np.arange(n_groups), side="right")
        grp_edge_start = node_e0[grp_node_start]
        grp_edge_end = np.where(
            grp_node_end < NPC, node_e0.take(grp_node_end, mode="clip"), e1 - e0
        )
        assert (grp_edge_end - grp_edge_start).max() <= GE
        assert (grp_node_end - grp_node_start).max() <= NODE_SLOTS
        cores.append(
            dict(k=k, e0=e0, e1=e1, n_groups=n_groups, d_k=d_k,
                 grp_node_start=grp_node_start, grp_node_end=grp_node_end,
                 grp_edge_start=grp_edge_start, grp_edge_end=grp_edge_end)
        )
        max_groups = max(max_groups, n_groups)

    n_super = -(-max_groups // LANES)

    # inp^T tiles: [core, sg, 96, 1024]; pair a cols a*512.., lane half rows 48*h..
    inpT_in = np.zeros((N_CORES, n_super, 2 * D_IN, 2 * GE), dtype=BF16)
    idx_in = np.zeros((N_CORES, n_super, P, NODE_SLOTS // 16), dtype=np.int16)
    # (flattened later to [core, 128, n_super*4] for the single preload DMA)

    jj = np.arange(NODE_SLOTS)
    for c in cores:
        k = c["k"]
        e0 = c["e0"]
        col_k = col_s[e0:c["e1"]]
        attr_k = attr_s[e0:c["e1"]]
        for gi in range(c["n_groups"]):
            sg, l = divmod(gi, LANES)
            pair, half = divmod(l, 2)
            es, ee = int(c["grp_edge_start"][gi]), int(c["grp_edge_end"][gi])
            cnt = ee - es
            blk = inpT_in[k, sg, half * D_IN:(half + 1) * D_IN,
                          pair * GE:pair * GE + cnt]
            blk[:D_NODE] = x[col_k[es:ee]].T.astype(BF16)
            blk[D_NODE:] = attr_k[es:ee].T.astype(BF16)
            ns, ne = int(c["grp_node_start"][gi]), int(c["grp_node_end"][gi])
            dloc = c["d_k"][ns:ne]
            last = np.maximum(np.cumsum(dloc) - 1, 0).astype(np.int16)
            nn = ne - ns
            j = jj[:nn]
            idx_in[k, sg, 32 * l + (j % 16), j // 16] = last
            idx_in[k, sg, 32 * l + 16 + (j % 16), j // 16] = last

    b1_tile = np.tile(np.asarray(b1, dtype=np.float32)[:, None], (2, 1))  # [128,1]
    W1blk = np.zeros((2 * D_IN, P), dtype=BF16)                            # [96, 128]
    W1blk[:D_IN, :D_HID] = W1
    W1blk[D_IN:, D_HID:] = W1
    W2blk = np.zeros((P, 2 * D_OUT), dtype=BF16)                           # [128, 64]
    W2blk[:D_HID, :D_OUT] = W2
    W2blk[D_HID:, D_OUT:] = W2

    return dict(
        cores=cores, n_super=n_super, inpT_in=inpT_in, idx_in=idx_in,
        b1_tile=b1_tile, W1blk=W1blk, W2blk=W2blk,
        deg=deg, b2=np.asarray(b2, dtype=np.float32),
    )


# ----------------------------------------------------------------------------
# numpy simulation of the HW dataflow (for correctness debugging)
# ----------------------------------------------------------------------------

def _simulate_hw(prep):
    n_super = prep["n_super"]
    W1blk, W2blk = prep["W1blk"], prep["W2blk"]
    b1t = prep["b1_tile"][:, 0]
    ext_all = np.zeros((N_CORES, n_super, P, NODE_SLOTS), dtype=np.float32)
    for k in range(N_CORES):
        for sg in range(n_super):
            inpT = prep["inpT_in"][k, sg].astype(np.float32)
            W1f = W1blk.astype(np.float32)
            W2f = W2blk.astype(np.float32)
            for pair in range(2):
                rhs = inpT[:, pair * GE:(pair + 1) * GE]      # [96, 512]
                h_pre = W1f.T @ rhs                            # [128, 512]
                h = np.maximum(h_pre + b1t[:, None], 0.0).astype(BF16).astype(np.float32)
                msg = W2f.T @ h                                # [64, 512]
                cum = np.cumsum(msg.astype(np.float64), axis=1).astype(np.float32)
                for half in range(2):
                    l = 2 * pair + half
                    idxw = prep["idx_in"][k, sg, 32 * l:32 * l + 16]
                    idx = np.zeros(NODE_SLOTS, dtype=np.int64)
                    for j in range(NODE_SLOTS):
                        idx[j] = idxw[j % 16, j // 16]
                    ext_all[k, sg, 32 * l:32 * l + 32, :] = \
                        cum[32 * half:32 * half + 32][:, idx]
    return ext_all


# ----------------------------------------------------------------------------
# assembly of the final output from extracted cumsums
# ----------------------------------------------------------------------------

def _assemble(prep, ext_all):
    out = np.zeros((N_NODES, D_OUT), dtype=np.float32)
    deg, b2 = prep["deg"], prep["b2"]
    for c in prep["cores"]:
        k = c["k"]
        for gi in range(c["n_groups"]):
            sg, l = divmod(gi, LANES)
            ns, ne = int(c["grp_node_start"][gi]), int(c["grp_node_end"][gi])
            nn = ne - ns
            v = ext_all[k, sg, 32 * l:32 * l + 32, :nn]
            dv = np.empty_like(v)
            dv[:, 0] = v[:, 0]
            dv[:, 1:] = v[:, 1:] - v[:, :-1]
            out[k * NPC + ns:k * NPC + ne] = dv.T
    out[deg == 0] = 0.0
    out += deg[:, None] * b2[None, :]
    return out


# ----------------------------------------------------------------------------
# bass kernel
# ----------------------------------------------------------------------------

def _build_bass(n_super):
    import concourse.bacc as bacc
    import concourse.mybir as mybir
    import concourse.tile as tile
    from concourse.tile_rust import add_dep_helper
    from contextlib import ExitStack

    nc = bacc.Bacc("TRN2", target_bir_lowering=False, debug=False,
                   enable_asserts=True, num_devices=N_CORES)
    f32 = mybir.dt.float32
    bf16 = mybir.dt.bfloat16
    inpT_d = nc.dram_tensor("inpT", [n_super, 2 * D_IN, 2 * GE], bf16, kind="ExternalInput").ap()
    idx_d = nc.dram_tensor("idx", [P, n_super * (NODE_SLOTS // 16)], mybir.dt.int16, kind="ExternalInput").ap()
    W1_d = nc.dram_tensor("W1blk", [2 * D_IN, P], bf16, kind="ExternalInput").ap()
    W2_d = nc.dram_tensor("W2blk", [P, 2 * D_OUT], bf16, kind="ExternalInput").ap()
    b1_d = nc.dram_tensor("b1t", [P, 1], f32, kind="ExternalInput").ap()
    ext_d = nc.dram_tensor("ext", [P, n_super * NODE_SLOTS], f32, kind="ExternalOutput").ap()

    with tile.TileContext(nc) as tc, ExitStack() as ctx:
        const = ctx.enter_context(tc.tile_pool(name="const", bufs=1))
        sb_in = ctx.enter_context(tc.tile_pool(name="sb_in", bufs=6))
        sb_h = ctx.enter_context(tc.tile_pool(name="sb_h", bufs=6))
        sb_out = ctx.enter_context(tc.tile_pool(name="sb_out", bufs=6))
        ps_h = ctx.enter_context(tc.tile_pool(name="ps_h", bufs=4, space="PSUM"))
        ps_m = ctx.enter_context(tc.tile_pool(name="ps_m", bufs=4, space="PSUM"))

        idx_all = const.tile([P, n_super * (NODE_SLOTS // 16)], mybir.dt.int16)
        nc.sync.dma_start(idx_all[:], idx_d[:])
        ones = const.tile([P, GE], f32)
        nc.gpsimd.memset(ones[:], 1.0)
        W1_s = const.tile([2 * D_IN, P], bf16)
        nc.sync.dma_start(W1_s[:], W1_d[:])
        W2_s = const.tile([P, 2 * D_OUT], bf16)
        nc.sync.dma_start(W2_s[:], W2_d[:])
        b1_s = const.tile([P, 1], f32)
        nc.sync.dma_start(b1_s[:], b1_d[:])

        ext_tiles = []
        pe_chain = []

        def chain(inst):
            if pe_chain:
                add_dep_helper(inst.ins, pe_chain[-1].ins, sync=False,
                               reason="PE weight-batch order")
            pe_chain.append(inst)

        BATCH = 1
        batches = [list(range(b, min(b + BATCH, n_super)))
                   for b in range(0, n_super, BATCH)]

        inps, msgs, hpres, hss = {}, {}, {}, {}

        def emit_w1(bi):
            for sg in batches[bi]:
                inpT_s = sb_in.tile([2 * D_IN, 2 * GE], bf16, tag="inpT",
                                    name=f"inp{sg}")
                nc.sync.dma_start(inpT_s[:], inpT_d[sg])
                inps[sg] = inpT_s
            for sg in batches[bi]:
                for pair in range(2):
                    h_pre = ps_h.tile([P, GE], f32, tag="hpre",
                                      name=f"hp{sg}_{pair}")
                    mm = nc.tensor.matmul(
                        h_pre[:], lhsT=W1_s[:],
                        rhs=inps[sg][:, pair * GE:(pair + 1) * GE],
                        start=True, stop=True,
                    )
                    chain(mm)
                    hpres[(sg, pair)] = h_pre
            for sg in batches[bi]:
                for pair in range(2):
                    h_s = sb_h.tile([P, GE], bf16, tag="hs",
                                    name=f"hs{sg}_{pair}")
                    nc.scalar.activation(
                        out=h_s[:], in_=hpres[(sg, pair)][:],
                        func=mybir.ActivationFunctionType.Relu, bias=b1_s[:],
                    )
                    hss[(sg, pair)] = h_s

        def emit_w2(bi):
            for sg in batches[bi]:
                msg_p = ps_m.tile([P, GE], f32, tag="msg", name=f"mp{sg}")
                for pair in range(2):
                    mm = nc.tensor.matmul(
                        msg_p[64 * pair:64 * pair + 64, :], lhsT=W2_s[:],
                        rhs=hss[(sg, pair)][:],
                        start=True, stop=True,
                    )
                    chain(mm)
                msgs[sg] = msg_p
            for sg in batches[bi]:
                cum_s = sb_out.tile([P, GE], f32, tag="cum", name=f"cum{sg}")
                nc.vector.tensor_tensor_scan(
                    out=cum_s[:], data0=ones[:], data1=msgs[sg][:], initial=0.0,
                    op0=mybir.AluOpType.mult, op1=mybir.AluOpType.add,
                )
                bi2 = sg % EXT_BATCH
                if bi2 == 0:
                    ext_s = sb_out.tile([P, EXT_BATCH * NODE_SLOTS], f32,
                                        tag="ext", name=f"ext{sg}")
                    ext_tiles.append(ext_s)
                ext_s = ext_tiles[-1]
                nc.gpsimd.ap_gather(
                    out_ap=ext_s[:, bi2 * NODE_SLOTS:(bi2 + 1) * NODE_SLOTS],
                    in_ap=cum_s[:],
                    idxs_ap=idx_all[:, sg * (NODE_SLOTS // 16):(sg + 1) * (NODE_SLOTS // 16)],
                    channels=P, num_elems=GE, d=1, num_idxs=NODE_SLOTS,
                )
                if bi2 == EXT_BATCH - 1 or sg == n_super - 1:
                    b0 = sg - bi2
                    nc.sync.dma_start(
                        ext_d[:, b0 * NODE_SLOTS:(sg + 1) * NODE_SLOTS],
                        ext_s[:, :(bi2 + 1) * NODE_SLOTS],
                    )

        LAG = 2
        for bi in range(len(batches)):
            emit_w1(bi)
            if bi >= LAG:
                emit_w2(bi - LAG)
        for bi in range(max(0, len(batches) - LAG), len(batches)):
            emit_w2(bi)

    nc.compile()
    return nc


def _run_hw(prep, trace=False):
    from concourse.bass_utils import run_bass_kernel_spmd

    n_super = prep["n_super"]
    if n_super not in _compiled_cache:
        _compiled_cache[n_super] = _build_bass(n_super)
    nc = _compiled_cache[n_super]

    in_maps = []
    for k in range(N_CORES):
        idx_flat = prep["idx_in"][k].transpose(1, 0, 2).reshape(P, -1)
        in_maps.append({
            "inpT": prep["inpT_in"][k],
            "idx": idx_flat,
            "W1blk": prep["W1blk"],
            "W2blk": prep["W2blk"],
            "b1t": prep["b1_tile"],
        })
    res = run_bass_kernel_spmd(nc, in_maps, list(range(N_CORES)), trace=trace)
    ext_all = np.stack([
        res.results[k]["ext"].reshape(P, n_super, NODE_SLOTS).transpose(1, 0, 2)
        for k in range(N_CORES)
    ])
    return ext_all, res


def kernel(x, edge_index, edge_attr, W1, b1, W2, b2, _numpy_sim=False):
    prep = _preprocess(x, edge_index, edge_attr, W1, b1, W2, b2)
    if _numpy_sim:
        ext_all = _simulate_hw(prep)
    else:
        ext_all, _ = _run_hw(prep)
    return _assemble(prep, ext_all)



# revision 10
# speedup vs baseline: 1.2831x; 1.2831x over previous
"""Trainium2 Bass kernel for GNN message passing (nn_MessageModel).

Reference computation:
    inp = concat([x[col], edge_attr], 1)          # [E, 48]
    h = relu(inp @ W1 + b1)                       # [E, 64]
    messages = h @ W2 + b2                        # [E, 32]
    out = segment_sum(messages, row, N)           # [N, 32]

Strategy (8 NeuronCores, SPMD, v2):
- Host: sort edges by destination row; shard into 8 equal 200k-edge
  chunks (nodes may straddle cores/lanes; host merges partial sums).
  Pack into perfect 512-edge lanes, 4 lanes per supergroup (sg), two
  sg per "block".  Feature-major inpT tiles [96, 2048] bf16 per block
  (two 48-row halves x two 512-col pairs per sg).  Pad edges use zero
  input (message == relu(b1)@W2 == m0, corrected on host).
- HW per block: one DMA loads inpT [96, 2048]; four block-diagonal W1
  matmuls -> hA2/hB2 PSUM [128, 2x512]; ReLU+bias split between ACT
  (hA2 full + right part of hB2) and DVE (left part of hB2) -> bf16;
  eight quadrant-tiled W2 matmuls -> msg2 PSUM [128, 1024] (4 lanes x
  32 feats x 2 sg); one DVE scan (bf16 ones operand -> full rate) over
  [128, 1024] -> chained per-partition-lane cumsum; one GPSIMD
  ap_gather extracts 80 node-boundary columns; ext DMA out every 2
  blocks.
- Host: per-chain adjacent differences of extracted cumsums,
  np.add.at merge of lane/core-straddling nodes, + deg * b2.
"""
import sys

if "/opt/trn_rl_repo" not in sys.path:
    sys.path.insert(0, "/opt/trn_rl_repo")

import numpy as np
import ml_dtypes

BF16 = ml_dtypes.bfloat16

N_NODES = 100000
N_EDGES = 1600000
D_NODE = 32
D_EDGE = 16
D_IN = D_NODE + D_EDGE
D_HID = 64
D_OUT = 32

N_CORES = 8
EPC = N_EDGES // N_CORES          # real edges per core (200000)
GE = 512                          # edges per lane
N_LANES = -(-EPC // GE) + (1 if (-(-EPC // GE)) % 8 else 0)  # 391 -> 392
N_LANES = ((EPC + GE - 1) // GE + 7) // 8 * 8                # 392 lanes
EPC_PAD = N_LANES * GE            # 200704 slots per core
N_BLK = N_LANES // 8              # 49 blocks (2 sg each)
NS = 40                           # extraction slots per lane
NW = 2 * NS                       # idx per window (80)
NWC = 8                           # idx columns reserved per window (16B aligned)
RELU_SPLIT = 176                  # cols of hB2 relu done on DVE (per sg)

_compiled_cache = {}


# ----------------------------------------------------------------------------
# host-side preprocessing
# ----------------------------------------------------------------------------

def _preprocess(x, edge_index, edge_attr, W1, b1, W2, b2):
    x = np.asarray(x, dtype=np.float32)
    W1 = np.asarray(W1, dtype=np.float32)
    b1 = np.asarray(b1, dtype=np.float32)
    W2 = np.asarray(W2, dtype=np.float32)
    b2 = np.asarray(b2, dtype=np.float32)
    row = np.asarray(edge_index[0], dtype=np.int64)
    col = np.asarray(edge_index[1], dtype=np.int64)
    order = np.argsort(row, kind="stable")
    row_s = row[order]
    col_s = col[order]
    attr_s = np.asarray(edge_attr, dtype=np.float32)[order]

    # message of a zero-input pad edge (b1 may be nonzero in general)
    m0 = np.maximum(b1, 0.0).astype(np.float32) @ W2  # [32]

    # ---- features, padded to [N_CORES, EPC_PAD, 48] ----
    feat = np.zeros((N_CORES, EPC_PAD, D_IN), dtype=np.float32)
    fx = x[col_s]                                    # [E, 32]
    feat[:, :EPC, :D_NODE] = fx.reshape(N_CORES, EPC, D_NODE)
    feat[:, :EPC, D_NODE:] = attr_s.reshape(N_CORES, EPC, D_EDGE)
    # inpT layout [core, blk, 96, 2048]:
    #   row = 48*half + f ; col = 1024*sgi + 512*pair + pos ;
    #   lane jj in block = 4*sgi + 2*pair + half
    v = feat.reshape(N_CORES, N_BLK, 2, 2, 2, GE, D_IN)
    #      [core, blk, sgi, pair, half, pos, f]
    inpT = np.ascontiguousarray(
        v.transpose(0, 1, 4, 6, 2, 3, 5)             # core blk half f sgi pair pos
    ).reshape(N_CORES, N_BLK, 2 * D_IN, 4 * GE).astype(BF16)

    # ---- per-core node-end bookkeeping ----
    rr = row_s.reshape(N_CORES, EPC)
    e_in_core = np.arange(EPC)
    pos_in_lane = (e_in_core % GE)
    lane_of = e_in_core // GE
    cores = []
    for k in range(N_CORES):
        re = rr[k]
        flag = np.empty(EPC, dtype=bool)
        flag[:-1] = (re[1:] != re[:-1]) | (pos_in_lane[:-1] == GE - 1)
        flag[-1] = True
        ends = np.nonzero(flag)[0]                  # edge indices of node-ends
        lanes = lane_of[ends]
        poss = pos_in_lane[ends]
        nodes = re[ends]
        # slot index within lane
        lane_start = np.searchsorted(lanes, np.arange(N_LANES), side="left")
        lane_end = np.searchsorted(lanes, np.arange(N_LANES), side="right")
        nn = lane_end - lane_start
        assert nn.max() <= NS, f"lane overflow: {nn.max()} > {NS}"
        si = np.arange(len(ends)) - lane_start[lanes]
        # lane decomposition
        w = lanes // 8
        jj = lanes % 8
        sgi = jj // 4
        j = jj % 4
        cores.append(dict(ends=ends, lanes=lanes, poss=poss, nodes=nodes,
                          si=si, w=w, sgi=sgi, j=j, nn=nn))

    # ---- idx tiles [core, 128, N_BLK*NWC] int16 (16B-aligned windows) ----
    idx_in = np.zeros((N_CORES, 128, N_BLK * NWC), dtype=np.int16)
    for k in range(N_CORES):
        c = cores[k]
        i_flat = NS * c["sgi"] + c["si"]             # [0, 80)
        val = (c["poss"] + GE * c["sgi"]).astype(np.int16)
        prow = 32 * c["j"] + (i_flat % 16)
        pcol = c["w"] * NWC + i_flat // 16
        idx_in[k, prow, pcol] = val
        idx_in[k, prow + 16, pcol] = val

    # ---- weights ----
    W1blk = np.zeros((2 * D_IN, 128), dtype=BF16)
    W1blk[:D_IN, :D_HID] = W1
    W1blk[D_IN:, D_HID:] = W1
    W2sb = np.zeros((128, D_OUT), dtype=BF16)
    W2sb[:D_HID] = W2
    W2sb[D_HID:] = W2
    b1t = np.tile(b1[:, None], (2, 1)).astype(np.float32)   # [128, 1]

    deg = np.bincount(row, minlength=N_NODES).astype(np.int64)

    return dict(cores=cores, inpT=inpT, idx_in=idx_in,
                W1blk=W1blk, W2sb=W2sb, b1t=b1t, m0=m0,
                deg=deg, b2=b2, W1=W1, b1=b1, W2=W2)


# ----------------------------------------------------------------------------
# numpy simulation of the HW dataflow (for correctness debugging)
# ----------------------------------------------------------------------------

def _simulate_hw(prep):
    W1f = prep["W1blk"].astype(np.float32)
    W2f = prep["W2sb"].astype(np.float32)
    b1t = prep["b1t"][:, 0]
    ext_all = np.zeros((N_CORES, 128, N_BLK * NW), dtype=np.float32)
    for k in range(N_CORES):
        for b in range(N_BLK):
            inpT = prep["inpT"][k, b].astype(np.float32)   # [96, 2048]
            msg2 = np.zeros((128, 2 * GE), dtype=np.float32)
            for sgi in range(2):
                for pair in range(2):
                    rhs = inpT[:, 1024 * sgi + 512 * pair:
                               1024 * sgi + 512 * pair + 512]
                    hp = W1f.T @ rhs                        # [128, 512]
                    h = np.maximum(hp + b1t[:, None], 0.0).astype(BF16)
                    h = h.astype(np.float32)
                    # lanes: half0 -> parts 0:64 -> msg parts 64*pair..,
                    #        half1 -> parts 64:128
                    lo = 64 * pair
                    msg2[lo:lo + 32, 512 * sgi:512 * sgi + 512] = \
                        W2f[:64, :].T @ h[:64]
                    msg2[lo + 32:lo + 64, 512 * sgi:512 * sgi + 512] = \
                        W2f[64:, :].T @ h[64:]
            cum = np.cumsum(msg2.astype(np.float64), axis=1).astype(np.float32)
            idxw = prep["idx_in"][k][:, b * NWC:b * NWC + NW // 16]
            for p in range(128):
                core16 = p // 16
                for i in range(NW):
                    ii = idxw[16 * core16 + (i % 16), i // 16]
                    ext_all[k, p, b * NW + i] = cum[p, ii]
    return ext_all


# ----------------------------------------------------------------------------
# assembly of the final output from extracted cumsums
# ----------------------------------------------------------------------------

def _assemble(prep, ext_all):
    # ext_all: [N_CORES, 128, N_BLK*NW]
    out = np.zeros((N_NODES, D_OUT), dtype=np.float32)
    m0 = prep["m0"]
    for k in range(N_CORES):
        c = prep["cores"][k]
        nE = len(c["ends"])
        # extraction values V [nE, 32]
        pcol = c["w"] * NW + NS * c["sgi"] + c["si"]
        prow = 32 * c["j"]
        V = ext_all[k][:, pcol]                      # [128, nE] -> rows
        V = np.stack([V[prow + f, np.arange(nE)] for f in range(32)], axis=1)
        # order ends by chain (w, j, sgi, si); within a chain prev = t-1
        key = ((c["w"] * 4 + c["j"]) * 2 + c["sgi"]) * (NS + 1) + c["si"]
        ordr = np.argsort(key, kind="stable")
        Vo = V[ordr]
        chain = (c["w"] * 4 + c["j"])[ordr]
        sgio = c["sgi"][ordr]
        sio = c["si"][ordr]
        first = np.empty(nE, dtype=bool)
        first[0] = True
        first[1:] = chain[1:] != chain[:-1]
        diffs = Vo.copy()
        diffs[~first] -= Vo[np.nonzero(~first)[0] - 1]
        # pad correction: first end of the sgi=1 lane in a chain whose
        # predecessor (sgi=0 lane of same chain) had a pad tail
        if m0.any():
            lane_real = np.bincount(c["lanes"],
                                    weights=None, minlength=N_LANES)
            # count real edges per lane
            realcnt = np.zeros(N_LANES, dtype=np.int64)
            np.add.at(realcnt, np.arange(EPC) // GE, 1)
            first_in_lane = sio == 0
            is_sg1 = sgio == 1
            tgt = first_in_lane & is_sg1
            lane0 = (chain // 4) * 8 + (chain % 4)   # sgi=0 lane of chain
            npad = GE - realcnt[lane0]
            diffs[tgt] -= npad[tgt][:, None] * m0[None, :]
            start_sg1 = tgt & first                  # chain starts at sgi=1
            # already covered by npad above (realcnt 0 -> 512 pads)
        nodes_o = c["nodes"][ordr]
        np.add.at(out, nodes_o, diffs)
    out += prep["deg"][:, None] * prep["b2"][None, :]
    return out


# ----------------------------------------------------------------------------
# bass kernel
# ----------------------------------------------------------------------------

def _build_bass():
    import concourse.bacc as bacc
    import concourse.mybir as mybir
    import concourse.tile as tile
    from contextlib import ExitStack

    nc = bacc.Bacc("TRN2", target_bir_lowering=False, debug=False,
                   enable_asserts=True, num_devices=N_CORES)
    f32 = mybir.dt.float32
    bf16 = mybir.dt.bfloat16
    inp_d = nc.dram_tensor("inpT", [N_BLK, 2 * D_IN, 4 * GE], bf16,
                           kind="ExternalInput").ap()
    idx_d = nc.dram_tensor("idx", [128, N_BLK * NWC], mybir.dt.int16,
                           kind="ExternalInput").ap()
    W1_d = nc.dram_tensor("W1blk", [2 * D_IN, 128], bf16,
                          kind="ExternalInput").ap()
    W2_d = nc.dram_tensor("W2sb", [128, D_OUT], bf16,
                          kind="ExternalInput").ap()
    b1_d = nc.dram_tensor("b1t", [128, 1], f32, kind="ExternalInput").ap()
    ext_d = nc.dram_tensor("ext", [128, N_BLK * NW], f32,
                           kind="ExternalOutput").ap()

    R = RELU_SPLIT

    with tile.TileContext(nc) as tc, ExitStack() as ctx:
        const = ctx.enter_context(tc.tile_pool(name="const", bufs=1))
        sb_in = ctx.enter_context(tc.tile_pool(name="sb_in", bufs=3))
        sb_h = ctx.enter_context(tc.tile_pool(name="sb_h", bufs=2))
        sb_out = ctx.enter_context(tc.tile_pool(name="sb_out", bufs=2))
        ps_h = ctx.enter_context(tc.tile_pool(name="ps_h", bufs=1,
                                              space="PSUM"))
        ps_m = ctx.enter_context(tc.tile_pool(name="ps_m", bufs=2,
                                              space="PSUM"))

        idx_all = const.tile([128, N_BLK * NWC], mybir.dt.int16)
        nc.sync.dma_start(idx_all[:], idx_d[:])
        ones = const.tile([128, 2 * GE], bf16)
        nc.gpsimd.memset(ones[:], 1.0)
        W1_s = const.tile([2 * D_IN, 128], bf16)
        nc.sync.dma_start(W1_s[:], W1_d[:])
        W2_s = const.tile([128, D_OUT], bf16)
        nc.sync.dma_start(W2_s[:], W2_d[:])
        b1_s = const.tile([128, 1], f32)
        nc.sync.dma_start(b1_s[:], b1_d[:])

        inps, hAs_t, hBs_t, msgs, cums = {}, {}, {}, {}, {}
        ext_tiles = {}

        def emit_dma(b):
            t = sb_in.tile([2 * D_IN, 4 * GE], bf16, tag="inp", name=f"inp{b}")
            nc.sync.dma_start(t[:], inp_d[b])
            inps[b] = t

        def emit_w1(b):
            hA2 = ps_h.tile([128, 2 * GE], f32, tag="hA", name=f"hA{b}")
            hB2 = ps_h.tile([128, 2 * GE], f32, tag="hB", name=f"hB{b}")
            t = inps[b]
            for sgi in range(2):
                nc.tensor.matmul(
                    hA2[:, 512 * sgi:512 * sgi + 512], lhsT=W1_s[:],
                    rhs=t[:, 1024 * sgi:1024 * sgi + 512],
                    start=True, stop=True)
            for sgi in range(2):
                nc.tensor.matmul(
                    hB2[:, 512 * sgi:512 * sgi + 512], lhsT=W1_s[:],
                    rhs=t[:, 1024 * sgi + 512:1024 * sgi + 1024],
                    start=True, stop=True)
            hAs_t[b] = (hA2, hB2)

        def emit_relu(b):
            hA2, hB2 = hAs_t[b]
            hAs = sb_h.tile([128, 2 * GE], bf16, tag="hAs", name=f"hAs{b}")
            hBs = sb_h.tile([128, 2 * GE], bf16, tag="hBs", name=f"hBs{b}")
            # ACT: all of hA2
            nc.scalar.activation(out=hAs[:], in_=hA2[:],
                                 func=mybir.ActivationFunctionType.Relu,
                                 bias=b1_s[:])
            # DVE: left R cols of each sg-half of hB2
            hB3 = hB2.rearrange("p (s g) -> p s g", s=2)
            hBs3 = hBs.rearrange("p (s g) -> p s g", s=2)
            nc.vector.tensor_scalar(
                out=hBs3[:, :, 0:R], in0=hB3[:, :, 0:R],
                scalar1=b1_s[:], scalar2=0.0,
                op0=mybir.AluOpType.add, op1=mybir.AluOpType.max)
            # ACT: right (GE-R) cols of each sg-half of hB2
            nc.scalar.activation(out=hBs3[:, :, R:GE], in_=hB3[:, :, R:GE],
                                 func=mybir.ActivationFunctionType.Relu,
                                 bias=b1_s[:])
            hBs_t[b] = (hAs, hBs)

        def emit_w2(b):
            hAs, hBs = hBs_t[b]
            msg2 = ps_m.tile([128, 2 * GE], f32, tag="msg", name=f"msg{b}")
            for sgi in range(2):
                c0 = 512 * sgi
                nc.tensor.matmul(msg2[0:32, c0:c0 + 512],
                                 lhsT=W2_s[0:64, :], rhs=hAs[0:64, c0:c0 + 512],
                                 start=True, stop=True, tile_position=(0, 0))
                nc.tensor.matmul(msg2[32:64, c0:c0 + 512],
                                 lhsT=W2_s[64:128, :], rhs=hAs[64:128, c0:c0 + 512],
                                 start=True, stop=True, tile_position=(64, 32))
                nc.tensor.matmul(msg2[64:96, c0:c0 + 512],
                                 lhsT=W2_s[0:64, :], rhs=hBs[0:64, c0:c0 + 512],
                                 start=True, stop=True, tile_position=(0, 64))
                nc.tensor.matmul(msg2[96:128, c0:c0 + 512],
                                 lhsT=W2_s[64:128, :], rhs=hBs[64:128, c0:c0 + 512],
                                 start=True, stop=True, tile_position=(64, 96))
            msgs[b] = msg2

        def emit_scan(b):
            cum = sb_out.tile([128, 2 * GE], f32, tag="cum", name=f"cum{b}")
            nc.vector.tensor_tensor_scan(
                out=cum[:], data0=ones[:], data1=msgs[b][:], initial=0.0,
                op0=mybir.AluOpType.mult, op1=mybir.AluOpType.add)
            cums[b] = cum

        def emit_gather(b):
            half = b % 2
            if half == 0:
                ext_s = sb_out.tile([128, 2 * NW], f32, tag="ext",
                                    name=f"ext{b}")
                ext_tiles[b] = ext_s
            ext_s = ext_tiles[b - half]
            nc.gpsimd.ap_gather(
                out_ap=ext_s[:, half * NW:(half + 1) * NW],
                in_ap=cums[b][:],
                idxs_ap=idx_all[:, b * NWC:b * NWC + NW // 16],
                channels=128, num_elems=2 * GE, d=1, num_idxs=NW)
            if half == 1 or b == N_BLK - 1:
                b0 = b - half
                nc.sync.dma_start(
                    ext_d[:, b0 * NW:(b + 1) * NW],
                    ext_s[:, :(half + 1) * NW])

        # software pipeline: DMA ahead, W2/scan/gather lag one block
        emit_dma(0)
        emit_dma(1)
        for b in range(N_BLK):
            if b + 2 < N_BLK:
                emit_dma(b + 2)
            emit_w1(b)
            emit_relu(b)
            if b > 0:
                emit_w2(b - 1)
                emit_scan(b - 1)
                emit_gather(b - 1)
        emit_w2(N_BLK - 1)
        emit_scan(N_BLK - 1)
        emit_gather(N_BLK - 1)

    nc.compile()
    return nc


def _run_hw(prep, trace=False):
    from concourse.bass_utils import run_bass_kernel_spmd

    if "nc" not in _compiled_cache:
        _compiled_cache["nc"] = _build_bass()
    nc = _compiled_cache["nc"]

    in_maps = []
    for k in range(N_CORES):
        in_maps.append({
            "inpT": prep["inpT"][k],
            "idx": prep["idx_in"][k],
            "W1blk": prep["W1blk"],
            "W2sb": prep["W2sb"],
            "b1t": prep["b1t"],
        })
    res = run_bass_kernel_spmd(nc, in_maps, list(range(N_CORES)), trace=trace)
    ext_all = np.stack([res.results[k]["ext"] for k in range(N_CORES)])
    return ext_all, res


def kernel(x, edge_index, edge_attr, W1, b1, W2, b2, _numpy_sim=False):
    prep = _preprocess(x, edge_index, edge_attr, W1, b1, W2, b2)
    if _numpy_sim:
        ext_all = _simulate_hw(prep)
    else:
        ext_all, _ = _run_hw(prep)
    return _assemble(prep, ext_all)
